# revision 39
# baseline (speedup 1.0000x reference)
"""Trainium2 Bass kernel for nn_DeepLatent loss (chamfer + L2 of a per-point MLP).

Strategy (8 cores, data-parallel over batch B=32 -> 4 samples/core):
  Per core, per sample s (channel-major layout: activations stored [C, Npoints]):
    h1 = relu(W1o.T @ obs^T + latbias)        precomputed on host (fp8)
    h2 = relu(W2.T @ h1 + b2)                 fp8 DoubleRow matmuls
    h3 = relu(W3.T @ h2 + b3)                 fp8 DoubleRow matmuls
    est = obs + W4.T @ h3 + b4                b4 folded into the PE accumulation
  Chamfer via a single augmented gram J = -d^2/2 (K=7 matmul):
    J[n,m] = gt_n . est_m - |est_m|^2/2 - |gt_n|^2/2
    dir1 (per gt):  ACT drains each PSUM tile through Exp(scale=BETA) with a
                    free-axis accumulator -> log-sum-exp soft row-max, so the
                    drain IS the reduction (host applies ln/BETA).
    dir2 (per est): running elementwise bf16 max across the 8 exp-space tiles
                    (max commutes with exp), then 8 PE transposes + one
                    3D-AP max-reduce; host takes ln/BETA.
  Relu chunks are split between ACT and DVE (tensor_scalar) to balance the two
  engines.  M1/M2/SES/cross partials are DMA'd out raw; host combines in f64.
"""

import ml_dtypes
import numpy as np
from contextlib import ExitStack

import concourse.bass as bass
import concourse.bacc as bacc
import concourse.mybir as mybir
import concourse.tile as tile
from concourse.bass_utils import run_bass_kernel_spmd


F32 = mybir.dt.float32
BF16 = mybir.dt.bfloat16
FP8 = mybir.dt.float8e4
AX = mybir.AxisListType
OP = mybir.AluOpType
ACTF = mybir.ActivationFunctionType
PM = mybir.MatmulPerfMode

B, N, L = 32, 1024, 256
NCORES = 8
BS = B // NCORES  # samples per core
NT = N // 128     # gram tiles per sample
BETA = 100.0
NEG = -3.0e38

# test.py hooks
TRACE = False
LAST = None

# which relu chunks go to DVE tensor_scalar instead of ACT (balance knobs)
L2_DVE = tuple(int(x) for x in __import__("os").environ.get("L2DVE", "2,3").split(",") if x != "")
L3_DVE = tuple(int(x) for x in __import__("os").environ.get("L3DVE", "1").split(",") if x != "")
import os as _os
NOPS = int(_os.environ.get("NOPS", "5"))
WARMN = int(_os.environ.get("WARMN", "60"))
PREROLL = int(_os.environ.get("PREROLL", "0"))
FILLN = int(_os.environ.get("FILLN", "0"))
FINDEFER = int(_os.environ.get("FINDEFER", "2"))


def build_program():
    nc = bacc.Bacc()

    obs_d = nc.dram_tensor("obs_t", [128, N], BF16, kind="ExternalInput")[:]
    ainit_d = nc.dram_tensor("a_init", [128, N], BF16, kind="ExternalInput")[:]
    cinit_d = nc.dram_tensor("c_init", [128, N], BF16, kind="ExternalInput")[:]
    obs2_d = nc.dram_tensor("obs_t2", [32, N], BF16, kind="ExternalInput")[:]
    ainit2_d = nc.dram_tensor("a_init2", [32, N], BF16, kind="ExternalInput")[:]
    cinit2_d = nc.dram_tensor("c_init2", [32, N], BF16, kind="ExternalInput")[:]
    h1_d = [nc.dram_tensor(f"h1in{i}", [128, 4, N], FP8, kind="ExternalInput")[:]
            for i in range(BS)]
    eye3d = nc.dram_tensor("eye34", [128, 3], BF16, kind="ExternalInput")[:]
    eye128d = nc.dram_tensor("eye128", [128, 128], BF16, kind="ExternalInput")[:]
    W2d = nc.dram_tensor("w2p", [128, 4, 512], FP8, kind="ExternalInput")[:]
    b2d = nc.dram_tensor("b2p", [128, 4], F32, kind="ExternalInput")[:]
    W3d = nc.dram_tensor("w3p", [128, 4, 256], FP8, kind="ExternalInput")[:]
    b3d = nc.dram_tensor("b3p", [128, 2], F32, kind="ExternalInput")[:]
    W4d = nc.dram_tensor("w4p", [128, 2, 3], BF16, kind="ExternalInput")[:]
    m1_d = nc.dram_tensor("m1", [128, NT * BS], F32, kind="ExternalOutput")[:]
    m2_d = nc.dram_tensor("m2", [128, NT * BS], F32, kind="ExternalOutput")[:]
    ct_d = nc.dram_tensor("ct_out", [128, N], BF16, kind="ExternalOutput")[:]
    ct2_d = nc.dram_tensor("ct2_out", [32, N], BF16, kind="ExternalOutput")[:]

    with tile.TileContext(nc) as tc, ExitStack() as ctx:
        singles = ctx.enter_context(tc.tile_pool(name="singles", bufs=1))

        def fixed(shape, name, dtype=F32):
            return singles.tile(shape, dtype, tag=name, name=name)

        # ---------- fixed tiles ----------
        eye34 = fixed([128, 3], "eye34", BF16)
        eye128 = fixed([128, 128], "eye128", BF16)
        w2t = fixed([128, 4, 512], "w2t", FP8)
        w3t = fixed([128, 4, 256], "w3t", FP8)
        w4t = fixed([128, 2, 3], "w4t", BF16)
        b2t = fixed([128, 4], "b2t")
        b3t = fixed([128, 2], "b3t")
        obsA = fixed([128, N], "obsA", BF16)
        At = fixed([128, N], "At", BF16)
        Ct = fixed([128, N], "Ct", BF16)
        obsA2 = fixed([32, N], "obsA2", BF16)
        At2 = fixed([32, N], "At2", BF16)
        Ct2 = fixed([32, N], "Ct2", BF16)

        def bandof(s):
            return (obsA, At, Ct, 32 * s) if s < 3 else (obsA2, At2, Ct2, 0)
        Jc_ = [fixed([128, N], f"Jc{i}", BF16) for i in range(NT)]
        R_ = [fixed([128, N], f"Rreg{i}", BF16) for i in range(2)]
        M1 = fixed([128, NT * BS], "M1")
        M2 = fixed([128, NT * BS], "M2")
        Ft = fixed([128, 8], "Ft")
        dumpx = fixed([128, N], "dumpx", BF16)
        dumpy = fixed([128, N], "dumpy", BF16)

        h1A = [fixed([128, 4, N], f"h1A{i}", FP8) for i in range(BS)]
        h2p = ctx.enter_context(tc.tile_pool(name="h2", bufs=2))
        h3p = ctx.enter_context(tc.tile_pool(name="h3", bufs=2))
        psA = ctx.enter_context(tc.tile_pool(name="psA", bufs=2, space="PSUM"))
        psG = ctx.enter_context(tc.tile_pool(name="psG", bufs=2, space="PSUM"))

        # ---------- startup ----------
        # HAM warm-up: ~4us of dummy matmuls (deps: one memset only) flip the
        # PE clock gate to 2.4GHz while the startup DMAs land
        fillw = fixed([128, 128], "fillw", BF16)
        nc.vector.memset(fillw, 0.001)
        fillps = psA.tile([128, 128], F32, tag="a", name="fillps")

        def filler(n):
            for _ in range(n):
                nc.tensor.matmul(fillps[0:1, 0:128], fillw[:, 0:1],
                                 fillw[:, 0:128], start=True, stop=True)
        filler(WARMN)
        for k in range(4):
            nc.scalar.dma_start(out=h1A[0][:, k, :], in_=h1_d[0][:, k, :])
        for q in range(2):
            nc.gpsimd.dma_start(out=w2t[:, 2 * q:2 * q + 2, :],
                                in_=W2d[:, 2 * q:2 * q + 2, :])
        nc.sync.dma_start(out=h1A[1], in_=h1_d[1])
        for q in range(2):
            nc.sync.dma_start(out=obsA[64 * q:64 * (q + 1), :],
                              in_=obs_d[64 * q:64 * (q + 1), :])
        nc.scalar.dma_start(out=h1A[2], in_=h1_d[2])
        nc.gpsimd.dma_start(out=h1A[3], in_=h1_d[3])
        nc.scalar.dma_start(out=obsA2, in_=obs2_d)
        nc.scalar.dma_start(out=b2t, in_=b2d)
        for q in range(2):
            nc.gpsimd.dma_start(out=w3t[:, 2 * q:2 * q + 2, :],
                                in_=W3d[:, 2 * q:2 * q + 2, :])
        for q in range(2):
            nc.sync.dma_start(out=At[64 * q:64 * (q + 1), :],
                              in_=ainit_d[64 * q:64 * (q + 1), :])
            nc.sync.dma_start(out=Ct[64 * q:64 * (q + 1), :],
                              in_=cinit_d[64 * q:64 * (q + 1), :])
        nc.sync.dma_start(out=At2, in_=ainit2_d)
        nc.sync.dma_start(out=Ct2, in_=cinit2_d)
        nc.scalar.dma_start(out=eye34, in_=eye3d)
        nc.scalar.dma_start(out=eye128, in_=eye128d)
        nc.gpsimd.dma_start(out=w4t, in_=W4d)
        nc.gpsimd.dma_start(out=b3t, in_=b3d)
        nc.vector.memset(Ft, 0.0)
        # trigger the ACT function-table load before real work arrives
        nc.scalar.activation(Ft[0:1, 0:8], Ft[0:1, 0:8], ACTF.Exp)

        # ---------- per-sample gram rounds (generator; interleaved with next MLP) ----------
        def gram_tiles(s):
            R = R_[s % 2]
            _, Ats, Cts, base = bandof(s)
            t0src = None
            for t in range(NT):
                gp = psG.tile([128, 1024], F32, tag="g", name=f"gp{s}_{t}")
                for j in range(2):
                    nc.tensor.matmul(
                        gp[:, 512 * j:512 * (j + 1)],
                        Ats[base:base + 7, 128 * t:128 * (t + 1)],
                        Cts[base:base + 7, 512 * j:512 * (j + 1)],
                        start=True, stop=True)
                    yield
                jc = Jc_[t]
                col = M1[:, NT * s + t:NT * s + t + 1]
                # exp-space drain: jc = exp(BETA*J); col = sum_est exp (LSE dir1)
                nc.scalar.activation(jc, gp[:, :], ACTF.Exp, scale=BETA,
                                     accum_out=col)
                if t == 0:
                    t0src = jc
                elif t == 1:
                    nc.vector.tensor_tensor(out=R, in0=jc, in1=t0src, op=OP.max)
                else:
                    nc.vector.tensor_tensor(out=R, in0=jc, in1=R, op=OP.max)
                yield

        def gram_finish(s):
            R = R_[s % 2]
            rt = psA.tile([128, NT, 128], BF16, tag="a", name=f"rt{s}")
            for k in range(NT):
                nc.tensor.transpose(rt[:, k, :], R[:, 128 * k:128 * (k + 1)],
                                    eye128)
            yield
            nc.vector.tensor_reduce(out=M2[:, NT * s:NT * (s + 1)],
                                    in_=rt[:, :, :], axis=AX.X, op=OP.max)
            nc.sync.dma_start(out=m1_d[:, NT * s:NT * (s + 1)],
                              in_=M1[:, NT * s:NT * (s + 1)])
            nc.sync.dma_start(out=m2_d[:, NT * s:NT * (s + 1)],
                              in_=M2[:, NT * s:NT * (s + 1)])
            yield

        def weave(gens, pattern):
            live = [iter(g) if g is not None else None for g in gens]
            while any(v is not None for v in live):
                for i in pattern:
                    if live[i] is not None:
                        try:
                            next(live[i])
                        except StopIteration:
                            live[i] = None
                            continue
                        yield

        def advance(it):
            if it is not None:
                next(it, None)

        # ---------- per-sample MLP, split by layer for paired interleave ----------
        h2ts = {}
        h3ts = {}

        def mlp_l2(s, hooks=None):
            h1t = h1A[s]
            h2t = h2p.tile([128, 4, N], FP8, tag="h2", name=f"h2_{s}")
            h2ts[s] = h2t
            for c in range(4):
                ps = psA.tile([128, N], F32, tag="a", name=f"l2ps{s}_{c}")
                for j in range(2):
                    for kp in range(2):
                        nc.tensor.matmul(
                            ps[:, 512 * j:512 * (j + 1)],
                            w2t[:, 2 * kp:2 * kp + 2, 128 * c:128 * (c + 1)],
                            h1t[:, 2 * kp:2 * kp + 2, 512 * j:512 * (j + 1)],
                            start=(kp == 0), stop=(kp == 1),
                            perf_mode=PM.DoubleRow)
                if c in L2_DVE:
                    nc.vector.tensor_scalar(
                        out=h2t[:, c, :], in0=ps[:, :], scalar1=b2t[:, c:c + 1],
                        scalar2=0.0, op0=OP.add, op1=OP.max)
                else:
                    nc.scalar.activation(h2t[:, c, :], ps[:, :], ACTF.Relu,
                                         bias=b2t[:, c:c + 1])

        def mlp_l3(s, hooks=None):
            h2t = h2ts[s]
            h3t = h3p.tile([128, 2, N], BF16, tag="h3", name=f"h3_{s}")
            h3ts[s] = h3t
            for c in range(2):
                ps = psA.tile([128, N], F32, tag="a", name=f"l3ps{s}_{c}")
                for j in range(2):
                    for kp in range(2):
                        nc.tensor.matmul(
                            ps[:, 512 * j:512 * (j + 1)],
                            w3t[:, 2 * kp:2 * kp + 2, 128 * c:128 * (c + 1)],
                            h2t[:, 2 * kp:2 * kp + 2, 512 * j:512 * (j + 1)],
                            start=(kp == 0), stop=(kp == 1),
                            perf_mode=PM.DoubleRow)
                if c in L3_DVE:
                    nc.vector.tensor_scalar(
                        out=h3t[:, c, :], in0=ps[:, :], scalar1=b3t[:, c:c + 1],
                        scalar2=0.0, op0=OP.add, op1=OP.max)
                else:
                    nc.scalar.activation(h3t[:, c, :], ps[:, :], ACTF.Relu,
                                         bias=b3t[:, c:c + 1])

        def mlp_l4(s, hooks=None):
            obsAs, Ats, Cts, base = bandof(s)
            obsT = obsAs[base:base + 4, :]  # 3 coord rows + ones row
            h3t = h3ts[s]
            ps4 = psA.tile([128, N], F32, tag="a", name=f"l4ps{s}")
            for j in range(2):
                for k in range(2):
                    nc.tensor.matmul(ps4[base:base + 3, 512 * j:512 * (j + 1)],
                                     w4t[:, k, :],
                                     h3t[:, k, 512 * j:512 * (j + 1)],
                                     start=(k == 0), stop=False)
                # obs + b4 folded via identity rows + bias row (ones in obsT)
                nc.tensor.matmul(ps4[base:base + 3, 512 * j:512 * (j + 1)],
                                 eye34[base:base + 4, :],
                                 obsT[:, 512 * j:512 * (j + 1)],
                                 start=False, stop=True)
            # est straight into the gram rhs band (same partitions as ps4)
            nc.vector.tensor_copy(out=Cts[base:base + 3, :],
                                  in_=ps4[base:base + 3, :])
            dx = dumpx if s % 2 == 0 else dumpy
            nc.vector.scalar_tensor_tensor(
                out=dx[base:base + 3, :], in0=Cts[base:base + 3, :],
                scalar=0.0, in1=Cts[base:base + 3, :], op0=OP.add, op1=OP.mult)
            nc.gpsimd.dma_start(out=Cts[base + 3:base + 6, :],
                                in_=dx[base:base + 3, :])
            if s < 3:
                nc.sync.dma_start(out=ct_d[base:base + 3, :],
                                  in_=Cts[base:base + 3, :])
            else:
                nc.sync.dma_start(out=ct2_d[0:3, :], in_=Cts[0:3, :])

        def _nops(n):
            for _ in range(n):
                yield

        def _chain(*gens):
            for g in gens:
                for _ in g:
                    yield

        # phase 1: all MLPs, layer-interleaved in pairs so one sample's
        # matmuls cover the other's relu latency at every layer boundary
        for pair in ((0, 1), (2, 3)):
            for s in pair:
                mlp_l2(s)
            for s in pair:
                mlp_l3(s)
            for s in pair:
                mlp_l4(s)
        # phase 2: all grams; defer each sample's finish (transposes + M2)
        # until 2 tiles into the next gram so it never stalls the PE queue
        fin = None
        for s in range(BS):
            cnt = 0
            for _ in gram_tiles(s):
                cnt += 1
                if fin is not None and cnt == FINDEFER:
                    for _ in fin:
                        pass
                    fin = None
            fin = gram_finish(s)
        for _ in fin:
            pass



    nc.compile()
    return nc


_program_cache = []


def kernel(**inputs):
    global LAST
    if not _program_cache:
        _program_cache.append(build_program())
    nc = _program_cache[0]

    def f32(x):
        return np.ascontiguousarray(np.asarray(x, dtype=np.float32))

    W1 = np.asarray(inputs["W1"], np.float32)
    W2 = np.asarray(inputs["W2"], np.float32)
    W3 = np.asarray(inputs["W3"], np.float32)
    W4 = np.asarray(inputs["W4"], np.float32)
    b1 = np.asarray(inputs["b1"], np.float32)
    b4 = np.asarray(inputs["b4"], np.float32)
    latent = np.asarray(inputs["latent"], np.float32)
    obs = np.asarray(inputs["obs"], np.float32)
    gt = np.asarray(inputs["obs_gt"], np.float32)

    # eye34 rows per band: [I3; b4] so the eye-matmul adds obs AND b4
    eye34 = np.zeros((128, 3), np.float32)
    for s in range(3):
        eye34[32 * s:32 * s + 3] = np.eye(3, dtype=np.float32)
        eye34[32 * s + 3] = b4
    # s=3 band lives at rows 0..3 of the [32,N] tiles but shares eye34 rows 0..3

    shared = {
        "eye34": np.ascontiguousarray(eye34.astype(ml_dtypes.bfloat16)),
        "eye128": np.eye(128, dtype=ml_dtypes.bfloat16),
        "w2p": np.ascontiguousarray(W2.reshape(4, 128, 512).transpose(1, 0, 2).astype(ml_dtypes.float8_e4m3)),
        "b2p": f32(np.asarray(inputs["b2"], np.float32).reshape(4, 128).T),
        "w3p": np.ascontiguousarray(W3.reshape(4, 128, 256).transpose(1, 0, 2).astype(ml_dtypes.float8_e4m3)),
        "b3p": f32(np.asarray(inputs["b3"], np.float32).reshape(2, 128).T),
        "w4p": np.ascontiguousarray(W4.reshape(2, 128, 3).transpose(1, 0, 2).astype(ml_dtypes.bfloat16)),
    }
    # layer 1 is a small fraction of the FLOPs: precompute on host
    lb_all = latent @ W1[3:, :] + b1  # [B, 512]
    h1_all = np.maximum(obs @ W1[0:3, :] + lb_all[:, None, :], 0.0)  # [B, N, 512]

    in_maps = []
    for c in range(NCORES):
        sl = slice(c * BS, (c + 1) * BS)
        m = dict(shared)
        for s in range(BS):
            m[f"h1in{s}"] = np.ascontiguousarray(
                h1_all[c * BS + s].T.reshape(4, 128, N).transpose(1, 0, 2)
                .astype(ml_dtypes.float8_e4m3))
        obsc = obs[sl]                    # [BS, N, 3]
        gtc = gt[sl]                      # [BS, N, 3]
        g2 = (gtc * gtc).sum(-1)          # [BS, N]
        O = np.zeros((160, N), np.float32)
        A = np.zeros((160, N), np.float32)
        C = np.zeros((160, N), np.float32)
        for s in range(BS):
            r = 32 * s if s < 3 else 128
            O[r:r + 3] = obsc[s].T
            O[r + 3] = 1.0
            A[r:r + 3] = gtc[s].T
            A[r + 3:r + 6] = -0.5
            A[r + 6] = -0.5 * g2[s]
            C[r + 6] = 1.0
        m["obs_t"] = np.ascontiguousarray(O[:128].astype(ml_dtypes.bfloat16))
        m["a_init"] = np.ascontiguousarray(A[:128].astype(ml_dtypes.bfloat16))
        m["c_init"] = np.ascontiguousarray(C[:128].astype(ml_dtypes.bfloat16))
        m["obs_t2"] = np.ascontiguousarray(O[128:].astype(ml_dtypes.bfloat16))
        m["a_init2"] = np.ascontiguousarray(A[128:].astype(ml_dtypes.bfloat16))
        m["c_init2"] = np.ascontiguousarray(C[128:].astype(ml_dtypes.bfloat16))
        in_maps.append(m)

    res = run_bass_kernel_spmd(nc, in_maps, core_ids=list(range(NCORES)),
                               trace=TRACE)
    LAST = res

    s_lnM1 = 0.0
    s_lnM2 = 0.0
    s_est2 = 0.0
    s_cross = 0.0
    for ci, r in enumerate(res.results):
        m1 = np.asarray(r["m1"], np.float64)
        m2 = np.asarray(r["m2"], np.float64)
        s_lnM1 += np.log(np.maximum(m1, 1e-38)).sum() / BETA
        s_lnM2 += np.log(np.maximum(m2, 1e-38)).sum() / BETA
        ct = np.asarray(r["ct_out"], np.float64)
        ct2 = np.asarray(r["ct2_out"], np.float64)
        for s in range(BS):
            base = 32 * s if s < 3 else 0
            estv = (ct[base:base + 3, :] if s < 3 else ct2[0:3, :])
            gts = gt[ci * BS + s].T.astype(np.float64)  # [3, N]
            s_est2 += (estv * estv).sum()
            s_cross += (gts * estv).sum()
    s_gt2 = float((gt.astype(np.float64) ** 2).sum())
    chm = (-2.0 * s_lnM1 - 2.0 * s_lnM2) / (B * N)
    l2 = (s_gt2 - 2.0 * s_cross + s_est2) / (B * N * 3)
    loss = 0.2 * chm + 0.8 * l2
    return np.asarray(loss, dtype=np.float32)


# revision 40
# speedup vs baseline: 1.0304x; 1.0304x over previous
"""Trainium2 Bass kernel for nn_DeepLatent loss (chamfer + L2 of a per-point MLP).

Strategy (8 cores, data-parallel over batch B=32 -> 4 samples/core):
  Per core, per sample s (channel-major layout: activations stored [C, Npoints]):
    h1 = relu(W1o.T @ obs^T + latbias)        precomputed on host (fp8)
    h2 = relu(W2.T @ h1 + b2)                 fp8 DoubleRow matmuls
    h3 = relu(W3.T @ h2 + b3)                 fp8 DoubleRow matmuls
    est = obs + W4.T @ h3 + b4                b4 folded into the PE accumulation
  Chamfer via a single augmented gram J = -d^2/2 (K=7 matmul):
    J[n,m] = gt_n . est_m - |est_m|^2/2 - |gt_n|^2/2
    dir1 (per gt):  ACT drains each PSUM tile through Exp(scale=BETA) with a
                    free-axis accumulator -> log-sum-exp soft row-max, so the
                    drain IS the reduction (host applies ln/BETA).
    dir2 (per est): running elementwise bf16 max across the 8 exp-space tiles
                    (max commutes with exp), then 8 PE transposes + one
                    3D-AP max-reduce; host takes ln/BETA.
  Relu chunks are split between ACT and DVE (tensor_scalar) to balance the two
  engines.  M1/M2/SES/cross partials are DMA'd out raw; host combines in f64.
"""

import ml_dtypes
import numpy as np
from contextlib import ExitStack

import concourse.bass as bass
import concourse.bacc as bacc
import concourse.mybir as mybir
import concourse.tile as tile
from concourse.bass_utils import run_bass_kernel_spmd


F32 = mybir.dt.float32
BF16 = mybir.dt.bfloat16
FP8 = mybir.dt.float8e4
AX = mybir.AxisListType
OP = mybir.AluOpType
ACTF = mybir.ActivationFunctionType
PM = mybir.MatmulPerfMode

B, N, L = 32, 1024, 256
NCORES = 8
BS = B // NCORES  # samples per core
NT = N // 128     # gram tiles per sample
BETA = 100.0
NEG = -3.0e38

# test.py hooks
TRACE = False
LAST = None

# which relu chunks go to DVE tensor_scalar instead of ACT (balance knobs)
L2_DVE = tuple(int(x) for x in __import__("os").environ.get("L2DVE", "2,3").split(",") if x != "")
L3_DVE = tuple(int(x) for x in __import__("os").environ.get("L3DVE", "1").split(",") if x != "")
import os as _os
NOPS = int(_os.environ.get("NOPS", "5"))
WARMN = int(_os.environ.get("WARMN", "60"))
PREROLL = int(_os.environ.get("PREROLL", "0"))
FILLN = int(_os.environ.get("FILLN", "0"))
FINDEFER = int(_os.environ.get("FINDEFER", "2"))


def build_program():
    nc = bacc.Bacc()

    obs_d = nc.dram_tensor("obs_t", [128, N], BF16, kind="ExternalInput")[:]
    ainit_d = nc.dram_tensor("a_init", [128, N], BF16, kind="ExternalInput")[:]
    cinit_d = nc.dram_tensor("c_init", [128, N], BF16, kind="ExternalInput")[:]
    obs2_d = nc.dram_tensor("obs_t2", [32, N], BF16, kind="ExternalInput")[:]
    ainit2_d = nc.dram_tensor("a_init2", [32, N], BF16, kind="ExternalInput")[:]
    cinit2_d = nc.dram_tensor("c_init2", [32, N], BF16, kind="ExternalInput")[:]
    h1_d = [nc.dram_tensor(f"h1in{i}", [128, 4, N], FP8, kind="ExternalInput")[:]
            for i in range(BS)]
    eye3d = nc.dram_tensor("eye34", [128, 3], BF16, kind="ExternalInput")[:]
    eye128d = nc.dram_tensor("eye128", [128, 128], BF16, kind="ExternalInput")[:]
    W2d = nc.dram_tensor("w2p", [128, 4, 512], FP8, kind="ExternalInput")[:]
    b2d = nc.dram_tensor("b2p", [128, 4], F32, kind="ExternalInput")[:]
    W3d = nc.dram_tensor("w3p", [128, 4, 256], FP8, kind="ExternalInput")[:]
    b3d = nc.dram_tensor("b3p", [128, 2], F32, kind="ExternalInput")[:]
    W4d = nc.dram_tensor("w4p", [128, 2, 3], BF16, kind="ExternalInput")[:]
    m1_d = nc.dram_tensor("m1", [128, NT * BS], F32, kind="ExternalOutput")[:]
    m2_d = nc.dram_tensor("m2", [128, NT * BS], F32, kind="ExternalOutput")[:]
    ct_d = nc.dram_tensor("ct_out", [128, N], BF16, kind="ExternalOutput")[:]
    ct2_d = nc.dram_tensor("ct2_out", [32, N], BF16, kind="ExternalOutput")[:]

    with tile.TileContext(nc) as tc, ExitStack() as ctx:
        singles = ctx.enter_context(tc.tile_pool(name="singles", bufs=1))

        def fixed(shape, name, dtype=F32):
            return singles.tile(shape, dtype, tag=name, name=name)

        # ---------- fixed tiles ----------
        eye34 = fixed([128, 3], "eye34", BF16)
        eye128 = fixed([128, 128], "eye128", BF16)
        w2t = fixed([128, 4, 512], "w2t", FP8)
        w3t = fixed([128, 4, 256], "w3t", FP8)
        w4t = fixed([128, 2, 3], "w4t", BF16)
        b2t = fixed([128, 4], "b2t")
        b3t = fixed([128, 2], "b3t")
        obsA = fixed([128, N], "obsA", BF16)
        At = fixed([128, N], "At", BF16)
        Ct = fixed([128, N], "Ct", BF16)
        obsA2 = fixed([32, N], "obsA2", BF16)
        At2 = fixed([32, N], "At2", BF16)
        Ct2 = fixed([32, N], "Ct2", BF16)

        def bandof(s):
            return (obsA, At, Ct, 32 * s) if s < 3 else (obsA2, At2, Ct2, 0)
        Jc_ = [fixed([128, N], f"Jc{i}", BF16) for i in range(NT)]
        R_ = [fixed([128, N], f"Rreg{i}", BF16) for i in range(2)]
        M1 = fixed([128, NT * BS], "M1")
        M2 = fixed([128, NT * BS], "M2")
        Ft = fixed([128, 8], "Ft")
        dumpx = fixed([128, N], "dumpx", BF16)
        dumpy = fixed([128, N], "dumpy", BF16)

        h1A = [fixed([128, 4, N], f"h1A{i}", FP8) for i in range(BS)]
        h2p = ctx.enter_context(tc.tile_pool(name="h2", bufs=2))
        h3p = ctx.enter_context(tc.tile_pool(name="h3", bufs=2))
        psA = ctx.enter_context(tc.tile_pool(name="psA", bufs=2, space="PSUM"))
        psG = ctx.enter_context(tc.tile_pool(name="psG", bufs=2, space="PSUM"))

        # ---------- startup ----------
        # HAM warm-up: ~4us of dummy matmuls (deps: one memset only) flip the
        # PE clock gate to 2.4GHz while the startup DMAs land
        fillw = fixed([128, 128], "fillw", BF16)
        nc.vector.memset(fillw, 0.001)
        fillps = psA.tile([128, 128], F32, tag="a", name="fillps")

        def filler(n):
            for _ in range(n):
                nc.tensor.matmul(fillps[0:1, 0:128], fillw[:, 0:1],
                                 fillw[:, 0:128], start=True, stop=True)
        filler(WARMN)
        for k in range(4):
            nc.scalar.dma_start(out=h1A[0][:, k, :], in_=h1_d[0][:, k, :])
        for q in range(2):
            nc.gpsimd.dma_start(out=w2t[:, 2 * q:2 * q + 2, :],
                                in_=W2d[:, 2 * q:2 * q + 2, :])
        nc.sync.dma_start(out=h1A[1], in_=h1_d[1])
        for q in range(2):
            nc.sync.dma_start(out=obsA[64 * q:64 * (q + 1), :],
                              in_=obs_d[64 * q:64 * (q + 1), :])
        nc.scalar.dma_start(out=h1A[2], in_=h1_d[2])
        nc.gpsimd.dma_start(out=h1A[3], in_=h1_d[3])
        nc.scalar.dma_start(out=obsA2, in_=obs2_d)
        nc.scalar.dma_start(out=b2t, in_=b2d)
        for q in range(2):
            nc.gpsimd.dma_start(out=w3t[:, 2 * q:2 * q + 2, :],
                                in_=W3d[:, 2 * q:2 * q + 2, :])
        for q in range(2):
            nc.sync.dma_start(out=At[64 * q:64 * (q + 1), :],
                              in_=ainit_d[64 * q:64 * (q + 1), :])
            nc.sync.dma_start(out=Ct[64 * q:64 * (q + 1), :],
                              in_=cinit_d[64 * q:64 * (q + 1), :])
        nc.sync.dma_start(out=At2, in_=ainit2_d)
        nc.sync.dma_start(out=Ct2, in_=cinit2_d)
        nc.scalar.dma_start(out=eye34, in_=eye3d)
        nc.scalar.dma_start(out=eye128, in_=eye128d)
        nc.gpsimd.dma_start(out=w4t, in_=W4d)
        nc.gpsimd.dma_start(out=b3t, in_=b3d)
        nc.vector.memset(Ft, 0.0)
        # trigger the ACT function-table load before real work arrives
        nc.scalar.activation(Ft[0:1, 0:8], Ft[0:1, 0:8], ACTF.Exp)

        # ---------- per-sample gram rounds (generator; interleaved with next MLP) ----------
        def gram_tiles(s):
            R = R_[s % 2]
            _, Ats, Cts, base = bandof(s)
            t0src = None
            for t in range(NT):
                gp = psG.tile([128, 1024], F32, tag="g", name=f"gp{s}_{t}")
                for j in range(2):
                    nc.tensor.matmul(
                        gp[:, 512 * j:512 * (j + 1)],
                        Ats[base:base + 7, 128 * t:128 * (t + 1)],
                        Cts[base:base + 7, 512 * j:512 * (j + 1)],
                        start=True, stop=True)
                    yield
                jc = Jc_[t]
                col = M1[:, NT * s + t:NT * s + t + 1]
                # exp-space drain: jc = exp(BETA*J); col = sum_est exp (LSE dir1)
                nc.scalar.activation(jc, gp[:, :], ACTF.Exp, scale=BETA,
                                     accum_out=col)
                if t == 0:
                    t0src = jc
                elif t == 1:
                    nc.vector.tensor_tensor(out=R, in0=jc, in1=t0src, op=OP.max)
                else:
                    nc.vector.tensor_tensor(out=R, in0=jc, in1=R, op=OP.max)
                yield

        def gram_finish(s):
            R = R_[s % 2]
            rt = psA.tile([128, NT, 128], BF16, tag="a", name=f"rt{s}")
            for k in range(NT):
                nc.tensor.transpose(rt[:, k, :], R[:, 128 * k:128 * (k + 1)],
                                    eye128)
            yield
            nc.vector.tensor_reduce(out=M2[:, NT * s:NT * (s + 1)],
                                    in_=rt[:, :, :], axis=AX.X, op=OP.max)
            nc.sync.dma_start(out=m1_d[:, NT * s:NT * (s + 1)],
                              in_=M1[:, NT * s:NT * (s + 1)])
            nc.sync.dma_start(out=m2_d[:, NT * s:NT * (s + 1)],
                              in_=M2[:, NT * s:NT * (s + 1)])
            yield

        def weave(gens, pattern):
            live = [iter(g) if g is not None else None for g in gens]
            while any(v is not None for v in live):
                for i in pattern:
                    if live[i] is not None:
                        try:
                            next(live[i])
                        except StopIteration:
                            live[i] = None
                            continue
                        yield

        def advance(it):
            if it is not None:
                next(it, None)

        # ---------- per-sample MLP, split by layer for paired interleave ----------
        h2ts = {}
        h3ts = {}

        def mlp_l2(s, hooks=None):
            h1t = h1A[s]
            h2t = h2p.tile([128, 4, N], FP8, tag="h2", name=f"h2_{s}")
            h2ts[s] = h2t
            for c in range(4):
                ps = psA.tile([128, N], F32, tag="a", name=f"l2ps{s}_{c}")
                for j in range(2):
                    for kp in range(2):
                        nc.tensor.matmul(
                            ps[:, 512 * j:512 * (j + 1)],
                            w2t[:, 2 * kp:2 * kp + 2, 128 * c:128 * (c + 1)],
                            h1t[:, 2 * kp:2 * kp + 2, 512 * j:512 * (j + 1)],
                            start=(kp == 0), stop=(kp == 1),
                            perf_mode=PM.DoubleRow)
                if c in L2_DVE:
                    nc.vector.tensor_scalar(
                        out=h2t[:, c, :], in0=ps[:, :], scalar1=b2t[:, c:c + 1],
                        scalar2=0.0, op0=OP.add, op1=OP.max)
                else:
                    nc.scalar.activation(h2t[:, c, :], ps[:, :], ACTF.Relu,
                                         bias=b2t[:, c:c + 1])

        def mlp_l3(s, hooks=None):
            h2t = h2ts[s]
            h3t = h3p.tile([128, 2, N], BF16, tag="h3", name=f"h3_{s}")
            h3ts[s] = h3t
            for c in range(2):
                ps = psA.tile([128, N], F32, tag="a", name=f"l3ps{s}_{c}")
                for j in range(2):
                    for kp in range(2):
                        nc.tensor.matmul(
                            ps[:, 512 * j:512 * (j + 1)],
                            w3t[:, 2 * kp:2 * kp + 2, 128 * c:128 * (c + 1)],
                            h2t[:, 2 * kp:2 * kp + 2, 512 * j:512 * (j + 1)],
                            start=(kp == 0), stop=(kp == 1),
                            perf_mode=PM.DoubleRow)
                if c in L3_DVE:
                    nc.vector.tensor_scalar(
                        out=h3t[:, c, :], in0=ps[:, :], scalar1=b3t[:, c:c + 1],
                        scalar2=0.0, op0=OP.add, op1=OP.max)
                else:
                    nc.scalar.activation(h3t[:, c, :], ps[:, :], ACTF.Relu,
                                         bias=b3t[:, c:c + 1])

        def mlp_l4(s, hooks=None):
            obsAs, Ats, Cts, base = bandof(s)
            obsT = obsAs[base:base + 4, :]  # 3 coord rows + ones row
            h3t = h3ts[s]
            ps4 = psA.tile([128, N], F32, tag="a", name=f"l4ps{s}")
            for j in range(2):
                for k in range(2):
                    nc.tensor.matmul(ps4[base:base + 3, 512 * j:512 * (j + 1)],
                                     w4t[:, k, :],
                                     h3t[:, k, 512 * j:512 * (j + 1)],
                                     start=(k == 0), stop=False)
                # obs + b4 folded via identity rows + bias row (ones in obsT)
                nc.tensor.matmul(ps4[base:base + 3, 512 * j:512 * (j + 1)],
                                 eye34[base:base + 4, :],
                                 obsT[:, 512 * j:512 * (j + 1)],
                                 start=False, stop=True)
            # est straight into the gram rhs band (same partitions as ps4)
            nc.vector.tensor_copy(out=Cts[base:base + 3, :],
                                  in_=ps4[base:base + 3, :])
            dx = dumpx if s % 2 == 0 else dumpy
            nc.vector.scalar_tensor_tensor(
                out=dx[base:base + 3, :], in0=Cts[base:base + 3, :],
                scalar=0.0, in1=Cts[base:base + 3, :], op0=OP.add, op1=OP.mult)
            nc.gpsimd.dma_start(out=Cts[base + 3:base + 6, :],
                                in_=dx[base:base + 3, :])
            if s < 3:
                nc.sync.dma_start(out=ct_d[base:base + 3, :],
                                  in_=Cts[base:base + 3, :])
            else:
                nc.sync.dma_start(out=ct2_d[0:3, :], in_=Cts[0:3, :])

        def _nops(n):
            for _ in range(n):
                yield

        def _chain(*gens):
            for g in gens:
                for _ in g:
                    yield

        # phase 1: all MLPs (PE-dense, relus on idle ACT/DVE)
        for s in range(BS):
            mlp_l2(s)
            mlp_l3(s)
            mlp_l4(s)
        # phase 2: all grams; defer each sample's finish (transposes + M2)
        # until 2 tiles into the next gram so it never stalls the PE queue
        fin = None
        for s in range(BS):
            cnt = 0
            for _ in gram_tiles(s):
                cnt += 1
                if fin is not None and cnt == FINDEFER:
                    for _ in fin:
                        pass
                    fin = None
            fin = gram_finish(s)
        for _ in fin:
            pass



    nc.compile()
    return nc


_program_cache = []


def kernel(**inputs):
    global LAST
    if not _program_cache:
        _program_cache.append(build_program())
    nc = _program_cache[0]

    def f32(x):
        return np.ascontiguousarray(np.asarray(x, dtype=np.float32))

    W1 = np.asarray(inputs["W1"], np.float32)
    W2 = np.asarray(inputs["W2"], np.float32)
    W3 = np.asarray(inputs["W3"], np.float32)
    W4 = np.asarray(inputs["W4"], np.float32)
    b1 = np.asarray(inputs["b1"], np.float32)
    b4 = np.asarray(inputs["b4"], np.float32)
    latent = np.asarray(inputs["latent"], np.float32)
    obs = np.asarray(inputs["obs"], np.float32)
    gt = np.asarray(inputs["obs_gt"], np.float32)

    # eye34 rows per band: [I3; b4] so the eye-matmul adds obs AND b4
    eye34 = np.zeros((128, 3), np.float32)
    for s in range(3):
        eye34[32 * s:32 * s + 3] = np.eye(3, dtype=np.float32)
        eye34[32 * s + 3] = b4
    # s=3 band lives at rows 0..3 of the [32,N] tiles but shares eye34 rows 0..3

    shared = {
        "eye34": np.ascontiguousarray(eye34.astype(ml_dtypes.bfloat16)),
        "eye128": np.eye(128, dtype=ml_dtypes.bfloat16),
        "w2p": np.ascontiguousarray(W2.reshape(4, 128, 512).transpose(1, 0, 2).astype(ml_dtypes.float8_e4m3)),
        "b2p": f32(np.asarray(inputs["b2"], np.float32).reshape(4, 128).T),
        "w3p": np.ascontiguousarray(W3.reshape(4, 128, 256).transpose(1, 0, 2).astype(ml_dtypes.float8_e4m3)),
        "b3p": f32(np.asarray(inputs["b3"], np.float32).reshape(2, 128).T),
        "w4p": np.ascontiguousarray(W4.reshape(2, 128, 3).transpose(1, 0, 2).astype(ml_dtypes.bfloat16)),
    }
    # layer 1 is a small fraction of the FLOPs: precompute on host
    lb_all = latent @ W1[3:, :] + b1  # [B, 512]
    h1_all = np.maximum(obs @ W1[0:3, :] + lb_all[:, None, :], 0.0)  # [B, N, 512]

    in_maps = []
    for c in range(NCORES):
        sl = slice(c * BS, (c + 1) * BS)
        m = dict(shared)
        for s in range(BS):
            m[f"h1in{s}"] = np.ascontiguousarray(
                h1_all[c * BS + s].T.reshape(4, 128, N).transpose(1, 0, 2)
                .astype(ml_dtypes.float8_e4m3))
        obsc = obs[sl]                    # [BS, N, 3]
        gtc = gt[sl]                      # [BS, N, 3]
        g2 = (gtc * gtc).sum(-1)          # [BS, N]
        O = np.zeros((160, N), np.float32)
        A = np.zeros((160, N), np.float32)
        C = np.zeros((160, N), np.float32)
        for s in range(BS):
            r = 32 * s if s < 3 else 128
            O[r:r + 3] = obsc[s].T
            O[r + 3] = 1.0
            A[r:r + 3] = gtc[s].T
            A[r + 3:r + 6] = -0.5
            A[r + 6] = -0.5 * g2[s]
            C[r + 6] = 1.0
        m["obs_t"] = np.ascontiguousarray(O[:128].astype(ml_dtypes.bfloat16))
        m["a_init"] = np.ascontiguousarray(A[:128].astype(ml_dtypes.bfloat16))
        m["c_init"] = np.ascontiguousarray(C[:128].astype(ml_dtypes.bfloat16))
        m["obs_t2"] = np.ascontiguousarray(O[128:].astype(ml_dtypes.bfloat16))
        m["a_init2"] = np.ascontiguousarray(A[128:].astype(ml_dtypes.bfloat16))
        m["c_init2"] = np.ascontiguousarray(C[128:].astype(ml_dtypes.bfloat16))
        in_maps.append(m)

    res = run_bass_kernel_spmd(nc, in_maps, core_ids=list(range(NCORES)),
                               trace=TRACE)
    LAST = res

    s_lnM1 = 0.0
    s_lnM2 = 0.0
    s_est2 = 0.0
    s_cross = 0.0
    for ci, r in enumerate(res.results):
        m1 = np.asarray(r["m1"], np.float64)
        m2 = np.asarray(r["m2"], np.float64)
        s_lnM1 += np.log(np.maximum(m1, 1e-38)).sum() / BETA
        s_lnM2 += np.log(np.maximum(m2, 1e-38)).sum() / BETA
        ct = np.asarray(r["ct_out"], np.float64)
        ct2 = np.asarray(r["ct2_out"], np.float64)
        for s in range(BS):
            base = 32 * s if s < 3 else 0
            estv = (ct[base:base + 3, :] if s < 3 else ct2[0:3, :])
            gts = gt[ci * BS + s].T.astype(np.float64)  # [3, N]
            s_est2 += (estv * estv).sum()
            s_cross += (gts * estv).sum()
    s_gt2 = float((gt.astype(np.float64) ** 2).sum())
    chm = (-2.0 * s_lnM1 - 2.0 * s_lnM2) / (B * N)
    l2 = (s_gt2 - 2.0 * s_cross + s_est2) / (B * N * 3)
    loss = 0.2 * chm + 0.8 * l2
    return np.asarray(loss, dtype=np.float32)


# revision 41
# speedup vs baseline: 1.1478x; 1.1139x over previous
"""Trainium2 Bass kernel for nn_DeepLatent loss (chamfer + L2 of a per-point MLP).

Strategy (8 cores, data-parallel over batch B=32 -> 4 samples/core):
  Per core, per sample s (channel-major layout: activations stored [C, Npoints]):
    h1 = relu(W1o.T @ obs^T + latbias)        precomputed on host (fp8)
    h2 = relu(W2.T @ h1 + b2)                 fp8 DoubleRow matmuls
    h3 = relu(W3.T @ h2 + b3)                 fp8 DoubleRow matmuls
    est = obs + W4.T @ h3 + b4                b4 folded into the PE accumulation
  Chamfer via a single augmented gram J = -d^2/2 (K=7 matmul):
    J[n,m] = gt_n . est_m - |est_m|^2/2 - |gt_n|^2/2
    dir1 (per gt):  ACT drains each PSUM tile through Exp(scale=BETA) with a
                    free-axis accumulator -> log-sum-exp soft row-max, so the
                    drain IS the reduction (host applies ln/BETA).
    dir2 (per est): running elementwise bf16 max across the 8 exp-space tiles
                    (max commutes with exp), then 8 PE transposes + one
                    3D-AP max-reduce; host takes ln/BETA.
  Relu chunks are split between ACT and DVE (tensor_scalar) to balance the two
  engines.  M1/M2/SES/cross partials are DMA'd out raw; host combines in f64.
"""

import ml_dtypes
import numpy as np
from contextlib import ExitStack

import concourse.bass as bass
import concourse.bacc as bacc
import concourse.mybir as mybir
import concourse.tile as tile
from concourse.bass_utils import run_bass_kernel_spmd


F32 = mybir.dt.float32
BF16 = mybir.dt.bfloat16
FP8 = mybir.dt.float8e4
AX = mybir.AxisListType
OP = mybir.AluOpType
ACTF = mybir.ActivationFunctionType
PM = mybir.MatmulPerfMode

B, N, L = 32, 1024, 256
NCORES = 8
BS = B // NCORES  # samples per core
NT = N // 128     # gram tiles per sample
BETA = 100.0
NEG = -3.0e38

# test.py hooks
TRACE = False
LAST = None

# which relu chunks go to DVE tensor_scalar instead of ACT (balance knobs)
L2_DVE = tuple(int(x) for x in __import__("os").environ.get("L2DVE", "2,3").split(",") if x != "")
L3_DVE = tuple(int(x) for x in __import__("os").environ.get("L3DVE", "1").split(",") if x != "")
import os as _os
NOPS = int(_os.environ.get("NOPS", "5"))
WARMN = int(_os.environ.get("WARMN", "60"))
PREROLL = int(_os.environ.get("PREROLL", "0"))
FILLN = int(_os.environ.get("FILLN", "0"))
FINDEFER = int(_os.environ.get("FINDEFER", "2"))


def build_program():
    nc = bacc.Bacc()

    obs_d = nc.dram_tensor("obs_t", [128, N], BF16, kind="ExternalInput")[:]
    ainit_d = nc.dram_tensor("a_init", [128, N], BF16, kind="ExternalInput")[:]
    cinit_d = nc.dram_tensor("c_init", [128, N], BF16, kind="ExternalInput")[:]
    obs2_d = nc.dram_tensor("obs_t2", [32, N], BF16, kind="ExternalInput")[:]
    ainit2_d = nc.dram_tensor("a_init2", [32, N], BF16, kind="ExternalInput")[:]
    cinit2_d = nc.dram_tensor("c_init2", [32, N], BF16, kind="ExternalInput")[:]
    h1_d = [nc.dram_tensor(f"h1in{i}", [128, 4, N], FP8, kind="ExternalInput")[:]
            for i in range(BS)]
    eye3d = nc.dram_tensor("eye34", [128, 3], BF16, kind="ExternalInput")[:]
    eye128d = nc.dram_tensor("eye128", [128, 128], BF16, kind="ExternalInput")[:]
    W2d = nc.dram_tensor("w2p", [128, 4, 512], FP8, kind="ExternalInput")[:]
    b2d = nc.dram_tensor("b2p", [128, 4], F32, kind="ExternalInput")[:]
    W3d = nc.dram_tensor("w3p", [128, 4, 256], FP8, kind="ExternalInput")[:]
    b3d = nc.dram_tensor("b3p", [128, 2], F32, kind="ExternalInput")[:]
    W4d = nc.dram_tensor("w4p", [128, 2, 3], BF16, kind="ExternalInput")[:]
    m1_d = nc.dram_tensor("m1", [128, NT * BS], F32, kind="ExternalOutput")[:]
    m2_d = nc.dram_tensor("m2", [128, NT * BS], F32, kind="ExternalOutput")[:]
    ct_d = nc.dram_tensor("ct_out", [128, N], BF16, kind="ExternalOutput")[:]
    ct2_d = nc.dram_tensor("ct2_out", [32, N], BF16, kind="ExternalOutput")[:]

    with tile.TileContext(nc) as tc, ExitStack() as ctx:
        singles = ctx.enter_context(tc.tile_pool(name="singles", bufs=1))

        def fixed(shape, name, dtype=F32):
            return singles.tile(shape, dtype, tag=name, name=name)

        # ---------- fixed tiles ----------
        eye34 = fixed([128, 3], "eye34", BF16)
        eye128 = fixed([128, 128], "eye128", BF16)
        w2t = fixed([128, 4, 512], "w2t", FP8)
        w3t = fixed([128, 4, 256], "w3t", FP8)
        w4t = fixed([128, 2, 3], "w4t", BF16)
        b2t = fixed([128, 4], "b2t")
        b3t = fixed([128, 2], "b3t")
        obsA = fixed([128, N], "obsA", BF16)
        At = fixed([128, N], "At", BF16)
        Ct = fixed([128, N], "Ct", BF16)
        obsA2 = fixed([32, N], "obsA2", BF16)
        At2 = fixed([32, N], "At2", BF16)
        Ct2 = fixed([32, N], "Ct2", BF16)

        def bandof(s):
            return (obsA, At, Ct, 32 * s) if s < 3 else (obsA2, At2, Ct2, 0)
        Jc_ = [fixed([128, N], f"Jc{i}", BF16) for i in range(NT)]
        R_ = [fixed([128, N], f"Rreg{i}", BF16) for i in range(2)]
        M1 = fixed([128, NT * BS], "M1")
        M2 = fixed([128, NT * BS], "M2")
        Ft = fixed([128, 8], "Ft")
        dumpx = fixed([128, N], "dumpx", BF16)
        dumpy = fixed([128, N], "dumpy", BF16)

        h1A = [fixed([128, 4, N], f"h1A{i}", FP8) for i in range(BS)]
        h2p = ctx.enter_context(tc.tile_pool(name="h2", bufs=2))
        h3p = ctx.enter_context(tc.tile_pool(name="h3", bufs=2))
        psA = ctx.enter_context(tc.tile_pool(name="psA", bufs=2, space="PSUM"))
        psG = ctx.enter_context(tc.tile_pool(name="psG", bufs=2, space="PSUM"))

        # ---------- startup ----------
        # HAM warm-up: ~4us of dummy matmuls (deps: one memset only) flip the
        # PE clock gate to 2.4GHz while the startup DMAs land
        fillw = fixed([128, 128], "fillw", BF16)
        nc.vector.memset(fillw, 0.001)
        fillps = psA.tile([128, 128], F32, tag="a", name="fillps")

        def filler(n):
            for _ in range(n):
                nc.tensor.matmul(fillps[0:1, 0:128], fillw[:, 0:1],
                                 fillw[:, 0:128], start=True, stop=True)
        filler(WARMN)
        for k in range(4):
            nc.scalar.dma_start(out=h1A[0][:, k, :], in_=h1_d[0][:, k, :])
        for q in range(2):
            nc.gpsimd.dma_start(out=w2t[:, 2 * q:2 * q + 2, :],
                                in_=W2d[:, 2 * q:2 * q + 2, :])
        nc.sync.dma_start(out=h1A[1], in_=h1_d[1])
        for q in range(2):
            nc.sync.dma_start(out=obsA[64 * q:64 * (q + 1), :],
                              in_=obs_d[64 * q:64 * (q + 1), :])
        nc.scalar.dma_start(out=h1A[2], in_=h1_d[2])
        nc.gpsimd.dma_start(out=h1A[3], in_=h1_d[3])
        nc.scalar.dma_start(out=obsA2, in_=obs2_d)
        nc.scalar.dma_start(out=b2t, in_=b2d)
        for q in range(2):
            nc.gpsimd.dma_start(out=w3t[:, 2 * q:2 * q + 2, :],
                                in_=W3d[:, 2 * q:2 * q + 2, :])
        for q in range(2):
            nc.sync.dma_start(out=At[64 * q:64 * (q + 1), :],
                              in_=ainit_d[64 * q:64 * (q + 1), :])
            nc.sync.dma_start(out=Ct[64 * q:64 * (q + 1), :],
                              in_=cinit_d[64 * q:64 * (q + 1), :])
        nc.sync.dma_start(out=At2, in_=ainit2_d)
        nc.sync.dma_start(out=Ct2, in_=cinit2_d)
        nc.scalar.dma_start(out=eye34, in_=eye3d)
        nc.scalar.dma_start(out=eye128, in_=eye128d)
        nc.gpsimd.dma_start(out=w4t, in_=W4d)
        nc.gpsimd.dma_start(out=b3t, in_=b3d)
        nc.vector.memset(Ft, 0.0)
        # trigger the ACT function-table load before real work arrives
        nc.scalar.activation(Ft[0:1, 0:8], Ft[0:1, 0:8], ACTF.Exp)

        # ---------- per-sample gram rounds (generator; interleaved with next MLP) ----------
        def gram_tiles(s):
            R = R_[s % 2]
            _, Ats, Cts, base = bandof(s)
            t0src = None
            for t in range(NT):
                gp = psG.tile([128, 1024], F32, tag="g", name=f"gp{s}_{t}")
                for j in range(2):
                    nc.tensor.matmul(
                        gp[:, 512 * j:512 * (j + 1)],
                        Ats[base:base + 7, 128 * t:128 * (t + 1)],
                        Cts[base:base + 7, 512 * j:512 * (j + 1)],
                        start=True, stop=True)
                    yield
                jc = Jc_[t]
                col = M1[:, NT * s + t:NT * s + t + 1]
                # exp-space drain: jc = exp(BETA*J); col = sum_est exp (LSE dir1)
                nc.scalar.activation(jc, gp[:, :], ACTF.Exp, scale=BETA,
                                     accum_out=col)
                if t == 0:
                    t0src = jc
                elif t == 1:
                    nc.vector.tensor_tensor(out=R, in0=jc, in1=t0src, op=OP.max)
                else:
                    nc.vector.tensor_tensor(out=R, in0=jc, in1=R, op=OP.max)
                yield

        def gram_finish(s):
            R = R_[s % 2]
            rt = psA.tile([128, NT, 128], BF16, tag="a", name=f"rt{s}")
            for k in range(NT):
                nc.tensor.transpose(rt[:, k, :], R[:, 128 * k:128 * (k + 1)],
                                    eye128)
            yield
            nc.vector.tensor_reduce(out=M2[:, NT * s:NT * (s + 1)],
                                    in_=rt[:, :, :], axis=AX.X, op=OP.max)
            nc.sync.dma_start(out=m1_d[:, NT * s:NT * (s + 1)],
                              in_=M1[:, NT * s:NT * (s + 1)])
            nc.sync.dma_start(out=m2_d[:, NT * s:NT * (s + 1)],
                              in_=M2[:, NT * s:NT * (s + 1)])
            yield

        def weave(gens, pattern):
            live = [iter(g) if g is not None else None for g in gens]
            while any(v is not None for v in live):
                for i in pattern:
                    if live[i] is not None:
                        try:
                            next(live[i])
                        except StopIteration:
                            live[i] = None
                            continue
                        yield

        def advance(it):
            if it is not None:
                next(it, None)

        # ---------- per-sample MLP ----------
        def mlp(s, hooks):
            obsAs, Ats, Cts, base = bandof(s)
            obsT = obsAs[base:base + 4, :]  # 3 coord rows + ones row
            h1t = h1A[s]
            h2t = h2p.tile([128, 4, N], FP8, tag="h2", name=f"h2_{s}")
            for c in range(4):
                ps = psA.tile([128, N], F32, tag="a", name=f"l2ps{s}_{c}")
                for j in range(2):
                    for kp in range(2):
                        nc.tensor.matmul(
                            ps[:, 512 * j:512 * (j + 1)],
                            w2t[:, 2 * kp:2 * kp + 2, 128 * c:128 * (c + 1)],
                            h1t[:, 2 * kp:2 * kp + 2, 512 * j:512 * (j + 1)],
                            start=(kp == 0), stop=(kp == 1),
                            perf_mode=PM.DoubleRow)
                    advance(hooks)
                if c in L2_DVE:
                    nc.vector.tensor_scalar(
                        out=h2t[:, c, :], in0=ps[:, :], scalar1=b2t[:, c:c + 1],
                        scalar2=0.0, op0=OP.add, op1=OP.max)
                else:
                    nc.scalar.activation(h2t[:, c, :], ps[:, :], ACTF.Relu,
                                         bias=b2t[:, c:c + 1])
                advance(hooks)

            h3t = h3p.tile([128, 2, N], BF16, tag="h3", name=f"h3_{s}")
            for c in range(2):
                ps = psA.tile([128, N], F32, tag="a", name=f"l3ps{s}_{c}")
                for j in range(2):
                    for kp in range(2):
                        nc.tensor.matmul(
                            ps[:, 512 * j:512 * (j + 1)],
                            w3t[:, 2 * kp:2 * kp + 2, 128 * c:128 * (c + 1)],
                            h2t[:, 2 * kp:2 * kp + 2, 512 * j:512 * (j + 1)],
                            start=(kp == 0), stop=(kp == 1),
                            perf_mode=PM.DoubleRow)
                    advance(hooks)
                if c in L3_DVE:
                    nc.vector.tensor_scalar(
                        out=h3t[:, c, :], in0=ps[:, :], scalar1=b3t[:, c:c + 1],
                        scalar2=0.0, op0=OP.add, op1=OP.max)
                else:
                    nc.scalar.activation(h3t[:, c, :], ps[:, :], ACTF.Relu,
                                         bias=b3t[:, c:c + 1])
                advance(hooks)

            ps4 = psA.tile([128, N], F32, tag="a", name=f"l4ps{s}")
            for j in range(2):
                for k in range(2):
                    nc.tensor.matmul(ps4[base:base + 3, 512 * j:512 * (j + 1)],
                                     w4t[:, k, :],
                                     h3t[:, k, 512 * j:512 * (j + 1)],
                                     start=(k == 0), stop=False)
                # obs + b4 folded via identity rows + bias row (ones in obsT)
                nc.tensor.matmul(ps4[base:base + 3, 512 * j:512 * (j + 1)],
                                 eye34[base:base + 4, :],
                                 obsT[:, 512 * j:512 * (j + 1)],
                                 start=False, stop=True)
            advance(hooks)
            # est straight into the gram rhs band (same partitions as ps4)
            nc.vector.tensor_copy(out=Cts[base:base + 3, :],
                                  in_=ps4[base:base + 3, :])
            dx = dumpx if s % 2 == 0 else dumpy
            nc.vector.scalar_tensor_tensor(
                out=dx[base:base + 3, :], in0=Cts[base:base + 3, :],
                scalar=0.0, in1=Cts[base:base + 3, :], op0=OP.add, op1=OP.mult)
            nc.gpsimd.dma_start(out=Cts[base + 3:base + 6, :],
                                in_=dx[base:base + 3, :])
            if s < 3:
                nc.sync.dma_start(out=ct_d[base:base + 3, :],
                                  in_=Cts[base:base + 3, :])
            else:
                nc.sync.dma_start(out=ct2_d[0:3, :], in_=Cts[0:3, :])

        # phase 1: all MLPs (PE-dense, relus on idle ACT/DVE)
        for s in range(BS):
            mlp(s, None)
        # phase 2: all grams; defer each sample's finish (transposes + M2)
        # until 2 tiles into the next gram so it never stalls the PE queue
        fin = None
        for s in range(BS):
            cnt = 0
            for _ in gram_tiles(s):
                cnt += 1
                if fin is not None and cnt == FINDEFER:
                    for _ in fin:
                        pass
                    fin = None
            fin = gram_finish(s)
        for _ in fin:
            pass



    nc.compile()
    return nc


_program_cache = []


def kernel(**inputs):
    global LAST
    if not _program_cache:
        _program_cache.append(build_program())
    nc = _program_cache[0]

    def f32(x):
        return np.ascontiguousarray(np.asarray(x, dtype=np.float32))

    W1 = np.asarray(inputs["W1"], np.float32)
    W2 = np.asarray(inputs["W2"], np.float32)
    W3 = np.asarray(inputs["W3"], np.float32)
    W4 = np.asarray(inputs["W4"], np.float32)
    b1 = np.asarray(inputs["b1"], np.float32)
    b4 = np.asarray(inputs["b4"], np.float32)
    latent = np.asarray(inputs["latent"], np.float32)
    obs = np.asarray(inputs["obs"], np.float32)
    gt = np.asarray(inputs["obs_gt"], np.float32)

    # eye34 rows per band: [I3; b4] so the eye-matmul adds obs AND b4
    eye34 = np.zeros((128, 3), np.float32)
    for s in range(3):
        eye34[32 * s:32 * s + 3] = np.eye(3, dtype=np.float32)
        eye34[32 * s + 3] = b4
    # s=3 band lives at rows 0..3 of the [32,N] tiles but shares eye34 rows 0..3

    shared = {
        "eye34": np.ascontiguousarray(eye34.astype(ml_dtypes.bfloat16)),
        "eye128": np.eye(128, dtype=ml_dtypes.bfloat16),
        "w2p": np.ascontiguousarray(W2.reshape(4, 128, 512).transpose(1, 0, 2).astype(ml_dtypes.float8_e4m3)),
        "b2p": f32(np.asarray(inputs["b2"], np.float32).reshape(4, 128).T),
        "w3p": np.ascontiguousarray(W3.reshape(4, 128, 256).transpose(1, 0, 2).astype(ml_dtypes.float8_e4m3)),
        "b3p": f32(np.asarray(inputs["b3"], np.float32).reshape(2, 128).T),
        "w4p": np.ascontiguousarray(W4.reshape(2, 128, 3).transpose(1, 0, 2).astype(ml_dtypes.bfloat16)),
    }
    # layer 1 is a small fraction of the FLOPs: precompute on host
    lb_all = latent @ W1[3:, :] + b1  # [B, 512]
    h1_all = np.maximum(obs @ W1[0:3, :] + lb_all[:, None, :], 0.0)  # [B, N, 512]

    in_maps = []
    for c in range(NCORES):
        sl = slice(c * BS, (c + 1) * BS)
        m = dict(shared)
        for s in range(BS):
            m[f"h1in{s}"] = np.ascontiguousarray(
                h1_all[c * BS + s].T.reshape(4, 128, N).transpose(1, 0, 2)
                .astype(ml_dtypes.float8_e4m3))
        obsc = obs[sl]                    # [BS, N, 3]
        gtc = gt[sl]                      # [BS, N, 3]
        g2 = (gtc * gtc).sum(-1)          # [BS, N]
        O = np.zeros((160, N), np.float32)
        A = np.zeros((160, N), np.float32)
        C = np.zeros((160, N), np.float32)
        for s in range(BS):
            r = 32 * s if s < 3 else 128
            O[r:r + 3] = obsc[s].T
            O[r + 3] = 1.0
            A[r:r + 3] = gtc[s].T
            A[r + 3:r + 6] = -0.5
            A[r + 6] = -0.5 * g2[s]
            C[r + 6] = 1.0
        m["obs_t"] = np.ascontiguousarray(O[:128].astype(ml_dtypes.bfloat16))
        m["a_init"] = np.ascontiguousarray(A[:128].astype(ml_dtypes.bfloat16))
        m["c_init"] = np.ascontiguousarray(C[:128].astype(ml_dtypes.bfloat16))
        m["obs_t2"] = np.ascontiguousarray(O[128:].astype(ml_dtypes.bfloat16))
        m["a_init2"] = np.ascontiguousarray(A[128:].astype(ml_dtypes.bfloat16))
        m["c_init2"] = np.ascontiguousarray(C[128:].astype(ml_dtypes.bfloat16))
        in_maps.append(m)

    res = run_bass_kernel_spmd(nc, in_maps, core_ids=list(range(NCORES)),
                               trace=TRACE)
    LAST = res

    s_lnM1 = 0.0
    s_lnM2 = 0.0
    s_est2 = 0.0
    s_cross = 0.0
    for ci, r in enumerate(res.results):
        m1 = np.asarray(r["m1"], np.float64)
        m2 = np.asarray(r["m2"], np.float64)
        s_lnM1 += np.log(np.maximum(m1, 1e-38)).sum() / BETA
        s_lnM2 += np.log(np.maximum(m2, 1e-38)).sum() / BETA
        ct = np.asarray(r["ct_out"], np.float64)
        ct2 = np.asarray(r["ct2_out"], np.float64)
        for s in range(BS):
            base = 32 * s if s < 3 else 0
            estv = (ct[base:base + 3, :] if s < 3 else ct2[0:3, :])
            gts = gt[ci * BS + s].T.astype(np.float64)  # [3, N]
            s_est2 += (estv * estv).sum()
            s_cross += (gts * estv).sum()
    s_gt2 = float((gt.astype(np.float64) ** 2).sum())
    chm = (-2.0 * s_lnM1 - 2.0 * s_lnM2) / (B * N)
    l2 = (s_gt2 - 2.0 * s_cross + s_est2) / (B * N * 3)
    loss = 0.2 * chm + 0.8 * l2
    return np.asarray(loss, dtype=np.float32)


# revision 42
# speedup vs baseline: 1.1626x; 1.0129x over previous
"""Trainium2 Bass kernel for nn_DeepLatent loss (chamfer + L2 of a per-point MLP).

Strategy (8 cores, data-parallel over batch B=32 -> 4 samples/core):
  Per core, per sample s (channel-major layout: activations stored [C, Npoints]):
    h1 = relu(W1o.T @ obs^T + latbias)        precomputed on host (fp8)
    h2 = relu(W2.T @ h1 + b2)                 fp8 DoubleRow matmuls
    h3 = relu(W3.T @ h2 + b3)                 fp8 DoubleRow matmuls
    est = obs + W4.T @ h3 + b4                b4 folded into the PE accumulation
  Chamfer via a single augmented gram J = -d^2/2 (K=7 matmul):
    J[n,m] = gt_n . est_m - |est_m|^2/2 - |gt_n|^2/2
    dir1 (per gt):  ACT drains each PSUM tile through Exp(scale=BETA) with a
                    free-axis accumulator -> log-sum-exp soft row-max, so the
                    drain IS the reduction (host applies ln/BETA).
    dir2 (per est): running elementwise bf16 max across the 8 exp-space tiles
                    (max commutes with exp), then 8 PE transposes + one
                    3D-AP max-reduce; host takes ln/BETA.
  Schedule: PHASE-SEPARATED issue — all four MLPs first (PE-dense; relus split
  ACT/DVE; est/est^2 on DVE), then all four grams as one chain so the ACT queue
  is a pure Exp sequence at its ~1.2us/tile floor with no FIFO head-of-line
  blocking.  Each sample's gram finish (transposes + M2 reduce) is deferred two
  tiles into the next gram, with its PSUM target in the (then-idle) MLP pool.
  A ~60-matmul dummy burst before real work flips the PE HAM clock gate to
  2.4GHz during the startup DMA window.  M1/M2/est partials are DMA'd out
  per-sample; the host combines in f64 (ln/BETA for the LSE terms, exact
  est^2/cross from the est dump).
"""

import ml_dtypes
import numpy as np
from contextlib import ExitStack

import concourse.bass as bass
import concourse.bacc as bacc
import concourse.mybir as mybir
import concourse.tile as tile
from concourse.bass_utils import run_bass_kernel_spmd


F32 = mybir.dt.float32
BF16 = mybir.dt.bfloat16
FP8 = mybir.dt.float8e4
AX = mybir.AxisListType
OP = mybir.AluOpType
ACTF = mybir.ActivationFunctionType
PM = mybir.MatmulPerfMode

B, N, L = 32, 1024, 256
NCORES = 8
BS = B // NCORES  # samples per core
NT = N // 128     # gram tiles per sample
BETA = 100.0
NEG = -3.0e38

# test.py hooks
TRACE = False
LAST = None

# which relu chunks go to DVE tensor_scalar instead of ACT (balance knobs)
L2_DVE = tuple(int(x) for x in __import__("os").environ.get("L2DVE", "2,3").split(",") if x != "")
L3_DVE = tuple(int(x) for x in __import__("os").environ.get("L3DVE", "1").split(",") if x != "")
import os as _os
NOPS = int(_os.environ.get("NOPS", "5"))
WARMN = int(_os.environ.get("WARMN", "60"))
PREROLL = int(_os.environ.get("PREROLL", "0"))
FILLN = int(_os.environ.get("FILLN", "0"))
FINDEFER = int(_os.environ.get("FINDEFER", "2"))


def build_program():
    nc = bacc.Bacc()

    obs_d = nc.dram_tensor("obs_t", [128, N], BF16, kind="ExternalInput")[:]
    ainit_d = nc.dram_tensor("a_init", [128, N], BF16, kind="ExternalInput")[:]
    cinit_d = nc.dram_tensor("c_init", [128, N], BF16, kind="ExternalInput")[:]
    obs2_d = nc.dram_tensor("obs_t2", [32, N], BF16, kind="ExternalInput")[:]
    ainit2_d = nc.dram_tensor("a_init2", [32, N], BF16, kind="ExternalInput")[:]
    cinit2_d = nc.dram_tensor("c_init2", [32, N], BF16, kind="ExternalInput")[:]
    h1_d = [nc.dram_tensor(f"h1in{i}", [128, 4, N], FP8, kind="ExternalInput")[:]
            for i in range(BS)]
    eye3d = nc.dram_tensor("eye34", [128, 3], BF16, kind="ExternalInput")[:]
    eye128d = nc.dram_tensor("eye128", [128, 128], BF16, kind="ExternalInput")[:]
    W2d = nc.dram_tensor("w2p", [128, 4, 512], FP8, kind="ExternalInput")[:]
    b2d = nc.dram_tensor("b2p", [128, 4], F32, kind="ExternalInput")[:]
    W3d = nc.dram_tensor("w3p", [128, 4, 256], FP8, kind="ExternalInput")[:]
    b3d = nc.dram_tensor("b3p", [128, 2], F32, kind="ExternalInput")[:]
    W4d = nc.dram_tensor("w4p", [128, 2, 3], BF16, kind="ExternalInput")[:]
    m1_d = nc.dram_tensor("m1", [128, NT * BS], F32, kind="ExternalOutput")[:]
    m2_d = nc.dram_tensor("m2", [128, NT * BS], F32, kind="ExternalOutput")[:]
    ct_d = nc.dram_tensor("ct_out", [128, N], BF16, kind="ExternalOutput")[:]
    ct2_d = nc.dram_tensor("ct2_out", [32, N], BF16, kind="ExternalOutput")[:]

    with tile.TileContext(nc) as tc, ExitStack() as ctx:
        singles = ctx.enter_context(tc.tile_pool(name="singles", bufs=1))

        def fixed(shape, name, dtype=F32):
            return singles.tile(shape, dtype, tag=name, name=name)

        # ---------- fixed tiles ----------
        eye34 = fixed([128, 3], "eye34", BF16)
        eye128 = fixed([128, 128], "eye128", BF16)
        w2t = fixed([128, 4, 512], "w2t", FP8)
        w3t = fixed([128, 4, 256], "w3t", FP8)
        w4t = fixed([128, 2, 3], "w4t", BF16)
        b2t = fixed([128, 4], "b2t")
        b3t = fixed([128, 2], "b3t")
        obsA = fixed([128, N], "obsA", BF16)
        At = fixed([128, N], "At", BF16)
        Ct = fixed([128, N], "Ct", BF16)
        obsA2 = fixed([32, N], "obsA2", BF16)
        At2 = fixed([32, N], "At2", BF16)
        Ct2 = fixed([32, N], "Ct2", BF16)

        def bandof(s):
            return (obsA, At, Ct, 32 * s) if s < 3 else (obsA2, At2, Ct2, 0)
        Jc_ = [fixed([128, N], f"Jc{i}", BF16) for i in range(NT)]
        R_ = [fixed([128, N], f"Rreg{i}", BF16) for i in range(2)]
        M1 = fixed([128, NT * BS], "M1")
        M2 = fixed([128, NT * BS], "M2")
        Ft = fixed([128, 8], "Ft")
        dumpx = fixed([128, N], "dumpx", BF16)
        dumpy = fixed([128, N], "dumpy", BF16)

        h1A = [fixed([128, 4, N], f"h1A{i}", FP8) for i in range(BS)]
        h2p = ctx.enter_context(tc.tile_pool(name="h2", bufs=2))
        h3p = ctx.enter_context(tc.tile_pool(name="h3", bufs=2))
        psA = ctx.enter_context(tc.tile_pool(name="psA", bufs=2, space="PSUM"))
        psG = ctx.enter_context(tc.tile_pool(name="psG", bufs=2, space="PSUM"))

        # ---------- startup ----------
        # HAM warm-up: ~4us of dummy matmuls (deps: one memset only) flip the
        # PE clock gate to 2.4GHz while the startup DMAs land
        fillw = fixed([128, 128], "fillw", BF16)
        nc.vector.memset(fillw, 0.001)
        fillps = psA.tile([128, 128], F32, tag="a", name="fillps")

        def filler(n):
            for _ in range(n):
                nc.tensor.matmul(fillps[0:1, 0:128], fillw[:, 0:1],
                                 fillw[:, 0:128], start=True, stop=True)
        filler(WARMN)
        for k in range(4):
            nc.scalar.dma_start(out=h1A[0][:, k, :], in_=h1_d[0][:, k, :])
        for q in range(2):
            nc.gpsimd.dma_start(out=w2t[:, 2 * q:2 * q + 2, :],
                                in_=W2d[:, 2 * q:2 * q + 2, :])
        nc.sync.dma_start(out=h1A[1], in_=h1_d[1])
        for q in range(2):
            nc.sync.dma_start(out=obsA[64 * q:64 * (q + 1), :],
                              in_=obs_d[64 * q:64 * (q + 1), :])
        nc.scalar.dma_start(out=h1A[2], in_=h1_d[2])
        nc.gpsimd.dma_start(out=h1A[3], in_=h1_d[3])
        nc.scalar.dma_start(out=obsA2, in_=obs2_d)
        nc.scalar.dma_start(out=b2t, in_=b2d)
        for q in range(2):
            nc.gpsimd.dma_start(out=w3t[:, 2 * q:2 * q + 2, :],
                                in_=W3d[:, 2 * q:2 * q + 2, :])
        for q in range(2):
            nc.sync.dma_start(out=At[64 * q:64 * (q + 1), :],
                              in_=ainit_d[64 * q:64 * (q + 1), :])
            nc.sync.dma_start(out=Ct[64 * q:64 * (q + 1), :],
                              in_=cinit_d[64 * q:64 * (q + 1), :])
        nc.sync.dma_start(out=At2, in_=ainit2_d)
        nc.sync.dma_start(out=Ct2, in_=cinit2_d)
        nc.scalar.dma_start(out=eye34, in_=eye3d)
        nc.scalar.dma_start(out=eye128, in_=eye128d)
        nc.gpsimd.dma_start(out=w4t, in_=W4d)
        nc.gpsimd.dma_start(out=b3t, in_=b3d)
        nc.vector.memset(Ft, 0.0)
        # trigger the ACT function-table load before real work arrives
        nc.scalar.activation(Ft[0:1, 0:8], Ft[0:1, 0:8], ACTF.Exp)

        # ---------- per-sample gram rounds (generator; interleaved with next MLP) ----------
        def gram_tiles(s):
            R = R_[s % 2]
            _, Ats, Cts, base = bandof(s)
            t0src = None
            for t in range(NT):
                gp = psG.tile([128, 1024], F32, tag="g", name=f"gp{s}_{t}")
                for j in range(2):
                    nc.tensor.matmul(
                        gp[:, 512 * j:512 * (j + 1)],
                        Ats[base:base + 7, 128 * t:128 * (t + 1)],
                        Cts[base:base + 7, 512 * j:512 * (j + 1)],
                        start=True, stop=True)
                    yield
                jc = Jc_[t]
                col = M1[:, NT * s + t:NT * s + t + 1]
                # exp-space drain: jc = exp(BETA*J); col = sum_est exp (LSE dir1)
                nc.scalar.activation(jc, gp[:, :], ACTF.Exp, scale=BETA,
                                     accum_out=col)
                if t == 0:
                    t0src = jc
                elif t == 1:
                    nc.vector.tensor_tensor(out=R, in0=jc, in1=t0src, op=OP.max)
                else:
                    nc.vector.tensor_tensor(out=R, in0=jc, in1=R, op=OP.max)
                yield

        def gram_finish(s):
            R = R_[s % 2]
            rt = psA.tile([128, NT, 128], BF16, tag="a", name=f"rt{s}")
            for k in range(NT):
                nc.tensor.transpose(rt[:, k, :], R[:, 128 * k:128 * (k + 1)],
                                    eye128)
            yield
            nc.vector.tensor_reduce(out=M2[:, NT * s:NT * (s + 1)],
                                    in_=rt[:, :, :], axis=AX.X, op=OP.max)
            nc.sync.dma_start(out=m1_d[:, NT * s:NT * (s + 1)],
                              in_=M1[:, NT * s:NT * (s + 1)])
            nc.sync.dma_start(out=m2_d[:, NT * s:NT * (s + 1)],
                              in_=M2[:, NT * s:NT * (s + 1)])
            yield

        def weave(gens, pattern):
            live = [iter(g) if g is not None else None for g in gens]
            while any(v is not None for v in live):
                for i in pattern:
                    if live[i] is not None:
                        try:
                            next(live[i])
                        except StopIteration:
                            live[i] = None
                            continue
                        yield

        def advance(it):
            if it is not None:
                next(it, None)

        # ---------- per-sample MLP ----------
        def mlp(s, hooks):
            obsAs, Ats, Cts, base = bandof(s)
            obsT = obsAs[base:base + 4, :]  # 3 coord rows + ones row
            h1t = h1A[s]
            h2t = h2p.tile([128, 4, N], FP8, tag="h2", name=f"h2_{s}")
            for c in range(4):
                ps = psA.tile([128, N], F32, tag="a", name=f"l2ps{s}_{c}")
                for j in range(2):
                    for kp in range(2):
                        nc.tensor.matmul(
                            ps[:, 512 * j:512 * (j + 1)],
                            w2t[:, 2 * kp:2 * kp + 2, 128 * c:128 * (c + 1)],
                            h1t[:, 2 * kp:2 * kp + 2, 512 * j:512 * (j + 1)],
                            start=(kp == 0), stop=(kp == 1),
                            perf_mode=PM.DoubleRow)
                    advance(hooks)
                if c in L2_DVE:
                    nc.vector.tensor_scalar(
                        out=h2t[:, c, :], in0=ps[:, :], scalar1=b2t[:, c:c + 1],
                        scalar2=0.0, op0=OP.add, op1=OP.max)
                else:
                    nc.scalar.activation(h2t[:, c, :], ps[:, :], ACTF.Relu,
                                         bias=b2t[:, c:c + 1])
                advance(hooks)

            h3t = h3p.tile([128, 2, N], BF16, tag="h3", name=f"h3_{s}")
            for c in range(2):
                ps = psA.tile([128, N], F32, tag="a", name=f"l3ps{s}_{c}")
                for j in range(2):
                    for kp in range(2):
                        nc.tensor.matmul(
                            ps[:, 512 * j:512 * (j + 1)],
                            w3t[:, 2 * kp:2 * kp + 2, 128 * c:128 * (c + 1)],
                            h2t[:, 2 * kp:2 * kp + 2, 512 * j:512 * (j + 1)],
                            start=(kp == 0), stop=(kp == 1),
                            perf_mode=PM.DoubleRow)
                    advance(hooks)
                if c in L3_DVE:
                    nc.vector.tensor_scalar(
                        out=h3t[:, c, :], in0=ps[:, :], scalar1=b3t[:, c:c + 1],
                        scalar2=0.0, op0=OP.add, op1=OP.max)
                else:
                    nc.scalar.activation(h3t[:, c, :], ps[:, :], ACTF.Relu,
                                         bias=b3t[:, c:c + 1])
                advance(hooks)

            ps4 = psA.tile([128, N], F32, tag="a", name=f"l4ps{s}")
            for j in range(2):
                for k in range(2):
                    nc.tensor.matmul(ps4[base:base + 3, 512 * j:512 * (j + 1)],
                                     w4t[:, k, :],
                                     h3t[:, k, 512 * j:512 * (j + 1)],
                                     start=(k == 0), stop=False)
                # obs + b4 folded via identity rows + bias row (ones in obsT)
                nc.tensor.matmul(ps4[base:base + 3, 512 * j:512 * (j + 1)],
                                 eye34[base:base + 4, :],
                                 obsT[:, 512 * j:512 * (j + 1)],
                                 start=False, stop=True)
            advance(hooks)
            # est straight into the gram rhs band (same partitions as ps4)
            nc.vector.tensor_copy(out=Cts[base:base + 3, :],
                                  in_=ps4[base:base + 3, :])
            dx = dumpx if s % 2 == 0 else dumpy
            nc.vector.scalar_tensor_tensor(
                out=dx[base:base + 3, :], in0=Cts[base:base + 3, :],
                scalar=0.0, in1=Cts[base:base + 3, :], op0=OP.add, op1=OP.mult)
            nc.gpsimd.dma_start(out=Cts[base + 3:base + 6, :],
                                in_=dx[base:base + 3, :])
            if s < 3:
                nc.sync.dma_start(out=ct_d[base:base + 3, :],
                                  in_=Cts[base:base + 3, :])
            else:
                nc.sync.dma_start(out=ct2_d[0:3, :], in_=Cts[0:3, :])

        # phase 1: all MLPs (PE-dense, relus on idle ACT/DVE)
        for s in range(BS):
            mlp(s, None)
        # phase 2: all grams; defer each sample's finish (transposes + M2)
        # until 2 tiles into the next gram so it never stalls the PE queue
        fin = None
        for s in range(BS):
            cnt = 0
            for _ in gram_tiles(s):
                cnt += 1
                if fin is not None and cnt == FINDEFER:
                    for _ in fin:
                        pass
                    fin = None
            fin = gram_finish(s)
        for _ in fin:
            pass



    nc.compile()
    return nc


_program_cache = []


def kernel(**inputs):
    global LAST
    if not _program_cache:
        _program_cache.append(build_program())
    nc = _program_cache[0]

    def f32(x):
        return np.ascontiguousarray(np.asarray(x, dtype=np.float32))

    W1 = np.asarray(inputs["W1"], np.float32)
    W2 = np.asarray(inputs["W2"], np.float32)
    W3 = np.asarray(inputs["W3"], np.float32)
    W4 = np.asarray(inputs["W4"], np.float32)
    b1 = np.asarray(inputs["b1"], np.float32)
    b4 = np.asarray(inputs["b4"], np.float32)
    latent = np.asarray(inputs["latent"], np.float32)
    obs = np.asarray(inputs["obs"], np.float32)
    gt = np.asarray(inputs["obs_gt"], np.float32)

    # eye34 rows per band: [I3; b4] so the eye-matmul adds obs AND b4
    eye34 = np.zeros((128, 3), np.float32)
    for s in range(3):
        eye34[32 * s:32 * s + 3] = np.eye(3, dtype=np.float32)
        eye34[32 * s + 3] = b4
    # s=3 band lives at rows 0..3 of the [32,N] tiles but shares eye34 rows 0..3

    shared = {
        "eye34": np.ascontiguousarray(eye34.astype(ml_dtypes.bfloat16)),
        "eye128": np.eye(128, dtype=ml_dtypes.bfloat16),
        "w2p": np.ascontiguousarray(W2.reshape(4, 128, 512).transpose(1, 0, 2).astype(ml_dtypes.float8_e4m3)),
        "b2p": f32(np.asarray(inputs["b2"], np.float32).reshape(4, 128).T),
        "w3p": np.ascontiguousarray(W3.reshape(4, 128, 256).transpose(1, 0, 2).astype(ml_dtypes.float8_e4m3)),
        "b3p": f32(np.asarray(inputs["b3"], np.float32).reshape(2, 128).T),
        "w4p": np.ascontiguousarray(W4.reshape(2, 128, 3).transpose(1, 0, 2).astype(ml_dtypes.bfloat16)),
    }
    # layer 1 is a small fraction of the FLOPs: precompute on host
    lb_all = latent @ W1[3:, :] + b1  # [B, 512]
    h1_all = np.maximum(obs @ W1[0:3, :] + lb_all[:, None, :], 0.0)  # [B, N, 512]

    in_maps = []
    for c in range(NCORES):
        sl = slice(c * BS, (c + 1) * BS)
        m = dict(shared)
        for s in range(BS):
            m[f"h1in{s}"] = np.ascontiguousarray(
                h1_all[c * BS + s].T.reshape(4, 128, N).transpose(1, 0, 2)
                .astype(ml_dtypes.float8_e4m3))
        obsc = obs[sl]                    # [BS, N, 3]
        gtc = gt[sl]                      # [BS, N, 3]
        g2 = (gtc * gtc).sum(-1)          # [BS, N]
        O = np.zeros((160, N), np.float32)
        A = np.zeros((160, N), np.float32)
        C = np.zeros((160, N), np.float32)
        for s in range(BS):
            r = 32 * s if s < 3 else 128
            O[r:r + 3] = obsc[s].T
            O[r + 3] = 1.0
            A[r:r + 3] = gtc[s].T
            A[r + 3:r + 6] = -0.5
            A[r + 6] = -0.5 * g2[s]
            C[r + 6] = 1.0
        m["obs_t"] = np.ascontiguousarray(O[:128].astype(ml_dtypes.bfloat16))
        m["a_init"] = np.ascontiguousarray(A[:128].astype(ml_dtypes.bfloat16))
        m["c_init"] = np.ascontiguousarray(C[:128].astype(ml_dtypes.bfloat16))
        m["obs_t2"] = np.ascontiguousarray(O[128:].astype(ml_dtypes.bfloat16))
        m["a_init2"] = np.ascontiguousarray(A[128:].astype(ml_dtypes.bfloat16))
        m["c_init2"] = np.ascontiguousarray(C[128:].astype(ml_dtypes.bfloat16))
        in_maps.append(m)

    res = run_bass_kernel_spmd(nc, in_maps, core_ids=list(range(NCORES)),
                               trace=TRACE)
    LAST = res

    s_lnM1 = 0.0
    s_lnM2 = 0.0
    s_est2 = 0.0
    s_cross = 0.0
    for ci, r in enumerate(res.results):
        m1 = np.asarray(r["m1"], np.float64)
        m2 = np.asarray(r["m2"], np.float64)
        s_lnM1 += np.log(np.maximum(m1, 1e-38)).sum() / BETA
        s_lnM2 += np.log(np.maximum(m2, 1e-38)).sum() / BETA
        ct = np.asarray(r["ct_out"], np.float64)
        ct2 = np.asarray(r["ct2_out"], np.float64)
        for s in range(BS):
            base = 32 * s if s < 3 else 0
            estv = (ct[base:base + 3, :] if s < 3 else ct2[0:3, :])
            gts = gt[ci * BS + s].T.astype(np.float64)  # [3, N]
            s_est2 += (estv * estv).sum()
            s_cross += (gts * estv).sum()
    s_gt2 = float((gt.astype(np.float64) ** 2).sum())
    chm = (-2.0 * s_lnM1 - 2.0 * s_lnM2) / (B * N)
    l2 = (s_gt2 - 2.0 * s_cross + s_est2) / (B * N * 3)
    loss = 0.2 * chm + 0.8 * l2
    return np.asarray(loss, dtype=np.float32)
